# revision 3
# baseline (speedup 1.0000x reference)
"""Trainium2 Bass kernel for nn_MA_73478300500338 (retrieval_knn).

Pipeline (reference semantics):
  q = relu(query_embedding)                      [B, D]
  sim = cos(q, memory_keys); idx = top_k(sim, 32)
  mk = memory_keys[idx]
  qt = relu(q @ Wq + bq); mt = relu(mk @ Wm + bm)
  attended = sum_j mt[:, j, :]   (softmax over size-1 axis == 1)
  ma = LN(attended + qt) * gamma + beta
  out = [q, ma] @ Wc + bc                        [B, C]

Distribution (8 NeuronCores):
  Phase 1 (candidate scan): memory bank sharded 8x (12500 rows/core, padded
    to 13312). Keys are L2-normalized on host (ranking-invariant), scaled and
    cast to fp8e4m3. Each core computes all 256 queries x 13312 dots with
    fp8 DoubleRow matmuls (2 contraction rows/partition, 0.5 cyc/row), then
    selects candidates: Act copies PSUM->SBUF fp16, DVE does 3 contiguous
    pairwise-max rounds (group size 8) + Max8/MaxIndex per 2048-key unit.
    That yields top-8 groups-of-8 per unit = 56 groups/core/query.
  Host: merges 448 candidate groups/query, exactly rescores the top-48
    groups' members in fp32 (and any unit that reported a duplicated index
    near the cut, to be robust to fp16 value ties), picks the exact top-32.
    Host work is pure indexing + a small (256x~400x512) sgemm.
  Phase 2 (attention MLP): queries sharded 8x (32/core), bf16. Biases are
    folded into the contraction (augmented ones-row), the sum over the 32
    retrieved keys runs on the PE via a 0/1 selector matmul, layernorm on
    Act/DVE, output projection on PE.
"""

import os
import sys
import json

import numpy as np
import ml_dtypes

os.environ.setdefault("MYCRO_LOCAL_CACHE", "1")
if "/opt/trn_rl_repo" not in sys.path:
    sys.path.insert(0, "/opt/trn_rl_repo")

try:
    import jax as _jax
    _jax.config.update("jax_compilation_cache_dir", "/tmp/jax_cache_nn_ma")
    _jax.config.update("jax_persistent_cache_min_entry_size_bytes", -1)
    _jax.config.update("jax_persistent_cache_min_compile_time_secs", 0.5)
except Exception:
    pass

import bass_rust
import concourse.bass as bass
import concourse.bacc as bacc
import concourse.mybir as mybir
import concourse.tile as tile
from concourse.vector_clock import ScopedClock

# ---------------------------------------------------------------------------
# Workaround: this walrus build supports a single sync-wait per CTRL
# instruction, but Tile's stock tail drain carries one wait per busy
# processor. Split them into standalone single-wait instructions.
# ---------------------------------------------------------------------------


def _patched_drain_and_barrier(self, tick_clock, wait_clock):
    nc = self.nc
    with nc.discard():
        probe = nc.sync.drain()
        wait_clock.add_sem_waits(
            probe.ins, ScopedClock({None: tick_clock.global_clock})
        )
        j = json.loads(nc.instruction_to_json(probe.ins))
    waits = (j.get("sync_info") or {}).get("on_wait") or []
    for w in waits:
        sem = bass_rust.SemaphoreHandle(w["ant_name"], w["id"])
        assert w["wait_mode"] == "sem-ge-imm", w
        nc.sync.wait_ge(sem, w["wait_value"])
    nc.sync.drain()
    nc.all_engine_barrier()
    popped = nc._tile_sem_poison_stack.pop()
    assert popped is self._sem_poison
    nc.clear_and_free_semaphores(list(self.sems.allocated().values()))
    nc.all_engine_barrier()


tile.TileContext._drain_and_barrier = _patched_drain_and_barrier

# ---------------------------------------------------------------------------
# Problem shapes (hardcoded per spec)
# ---------------------------------------------------------------------------
B, N, D = 256, 100000, 512
AU, C, K = 256, 100, 32
NCORES = 8
SH = N // NCORES            # 12500 keys per core
SHP = 13312                 # padded shard width (13 x 1024, %16 == 0)
UNIT = 2048                 # selection unit width (keys)
NU = 7                      # units per core: 6 x 2048 + 1 x 1024
G = 8                       # group size (keys per candidate group)
TOPG = 48                   # groups rescored exactly per query
KSCALE = 64.0               # fp8 key scale (ranking-invariant)
EPS_LN = 1e-5

F32 = mybir.dt.float32
F16 = mybir.dt.float16
BF16 = mybir.dt.bfloat16
FP8 = mybir.dt.float8e4
U16 = mybir.dt.uint16
F8NP = ml_dtypes.float8_e4m3
BF16NP = ml_dtypes.bfloat16

_cache = {}


# ---------------------------------------------------------------------------
# Phase 1: fp8 DoubleRow dots + grouped top-8 candidates per unit
# ---------------------------------------------------------------------------


def _build_phase1():
    nc = bacc.Bacc()
    q8d = nc.dram_tensor("q8", [128, 2, 2, B], FP8, kind="ExternalInput")
    k8d = nc.dram_tensor("k8", [128, 2, 2, SHP], FP8, kind="ExternalInput")
    t8 = nc.dram_tensor("t8", [2, 128, NU, 8], F16, kind="ExternalOutput")
    i8 = nc.dram_tensor("i8", [2, 128, NU, 8], U16, kind="ExternalOutput")

    with tile.TileContext(nc) as tc:
        with (
            tc.tile_pool(name="persist", bufs=1) as persist,
            tc.tile_pool(name="work", bufs=3) as wp,
            tc.tile_pool(name="psum", bufs=2, space="PSUM") as psump,
        ):
            q8 = persist.tile([128, 2, 2, B], FP8)
            nc.sync.dma_start(out=q8, in_=q8d[:, :, :, :])
            k8 = persist.tile([128, 2, 2, SHP], FP8)
            for u in range(NU):
                lo, hi = u * UNIT, min((u + 1) * UNIT, SHP)
                nc.sync.dma_start(out=k8[:, :, :, lo:hi], in_=k8d[:, :, :, lo:hi])

            t8s = persist.tile([128, 2, NU, 8], F16)
            i8s = persist.tile([128, 2, NU, 8], U16)

            for u in range(NU):
                lo, hi = u * UNIT, min((u + 1) * UNIT, SHP)
                uw = hi - lo                       # 2048, last unit 1024
                ng = uw // G                       # groups in this unit
                for bc in range(2):
                    ps = psump.tile([128, UNIT], F32, tag="ps")
                    for w2 in range(uw // 512):
                        for kc in range(2):
                            nc.tensor.matmul(
                                ps[:, w2 * 512:(w2 + 1) * 512],
                                q8[:, kc, :, bc * 128:(bc + 1) * 128],
                                k8[:, kc, :, lo + w2 * 512:lo + (w2 + 1) * 512],
                                start=(kc == 0), stop=(kc == 1),
                                perf_mode=mybir.MatmulPerfMode.DoubleRow,
                            )
                    dw = wp.tile([128, UNIT], F16, tag="dw")
                    nc.scalar.copy(out=dw[:, :uw], in_=ps[:, :uw])
                    h = uw // 2
                    t1 = wp.tile([128, UNIT // 2], F16, tag="t1")
                    nc.vector.tensor_max(out=t1[:, :h], in0=dw[:, :h], in1=dw[:, h:uw])
                    t2 = wp.tile([128, UNIT // 4], F16, tag="t2")
                    nc.vector.tensor_max(out=t2[:, :h // 2], in0=t1[:, :h // 2],
                                         in1=t1[:, h // 2:h])
                    g8 = wp.tile([128, UNIT // 8], F16, tag="g8")
                    nc.vector.tensor_max(out=g8[:, :ng], in0=t2[:, :ng],
                                         in1=t2[:, ng:2 * ng])
                    nc.vector.max(out=t8s[:, bc, u, :], in_=g8[:, :ng])
                    nc.vector.max_index(out=i8s[:, bc, u, :],
                                        in_max=t8s[:, bc, u, :], in_values=g8[:, :ng])

            for bc in range(2):
                nc.sync.dma_start(out=t8[bc, :, :, :], in_=t8s[:, bc, :, :])
                nc.sync.dma_start(out=i8[bc, :, :, :], in_=i8s[:, bc, :, :])
    nc.finalize()
    return nc


# ---------------------------------------------------------------------------
# Phase 2: attention MLP + LN + output projection (32 queries per core, bf16)
# ---------------------------------------------------------------------------
BQ = B // NCORES            # 32 queries per core
NK = BQ * K                 # 1024 gathered key columns per core
DC5 = 5                     # 4 d-chunks + 1 bias-aug chunk
NKC = NK // 128             # 8 nk chunks


def _build_phase2():
    nc = bacc.Bacc()
    mk = nc.dram_tensor("mk", [128, DC5, NK], BF16, kind="ExternalInput")
    qr_ = nc.dram_tensor("qr", [128, DC5, BQ], BF16, kind="ExternalInput")
    wq_ = nc.dram_tensor("wq", [128, DC5, AU], BF16, kind="ExternalInput")
    wm_ = nc.dram_tensor("wm", [128, DC5, AU], BF16, kind="ExternalInput")
    wc_ = nc.dram_tensor("wc", [128, 7, AU], BF16, kind="ExternalInput")
    sel_ = nc.dram_tensor("sel", [128, NKC, BQ], BF16, kind="ExternalInput")
    gam = nc.dram_tensor("gam", [AU], F32, kind="ExternalInput")
    bet = nc.dram_tensor("bet", [AU], F32, kind="ExternalInput")
    ident = nc.dram_tensor("ident", [128, 128], F32, kind="ExternalInput")
    out = nc.dram_tensor("out", [BQ, C], F32, kind="ExternalOutput")

    with tile.TileContext(nc) as tc:
        with (
            tc.tile_pool(name="p", bufs=1) as pool,
            tc.tile_pool(name="psmt", bufs=2, space="PSUM") as psmt,
            tc.tile_pool(name="ps1", bufs=1, space="PSUM") as ps1,
        ):
            # loads: mt operands first so the PE starts ASAP
            wm = pool.tile([128, DC5, AU], BF16)
            nc.sync.dma_start(out=wm, in_=wm_[:, :, :])
            mkt = pool.tile([128, DC5, NK], BF16)
            for h in range(2):
                nc.sync.dma_start(
                    out=mkt[:, :, h * (NK // 2):(h + 1) * (NK // 2)],
                    in_=mk[:, :, h * (NK // 2):(h + 1) * (NK // 2)])
            sel = pool.tile([128, NKC, BQ], BF16)
            nc.sync.dma_start(out=sel, in_=sel_[:, :, :])
            qr = pool.tile([128, DC5, BQ], BF16)
            nc.sync.dma_start(out=qr, in_=qr_[:, :, :])
            wq = pool.tile([128, DC5, AU], BF16)
            nc.sync.dma_start(out=wq, in_=wq_[:, :, :])
            wc = pool.tile([128, 7, AU], BF16)
            nc.sync.dma_start(out=wc, in_=wc_[:, :, :])
            grow = pool.tile([BQ, AU], F32)
            nc.sync.dma_start(out=grow, in_=bass.AP(gam, 0, [[0, BQ], [1, AU]]))
            brow = pool.tile([BQ, AU], F32)
            nc.sync.dma_start(out=brow, in_=bass.AP(bet, 0, [[0, BQ], [1, AU]]))
            idt = pool.tile([128, 128], F32)
            nc.sync.dma_start(out=idt, in_=ident[:, :])

            # mt[nk, au] = relu(mk_aug^T @ Wm_aug), nk-chunked
            mt = pool.tile([128, NKC, AU], BF16)
            for k_ in range(NKC):
                pmt = psmt.tile([128, AU], F32, tag="pmt")
                for c in range(DC5):
                    nc.tensor.matmul(
                        pmt, mkt[:, c, k_ * 128:(k_ + 1) * 128], wm[:, c, :],
                        start=(c == 0), stop=(c == DC5 - 1))
                nc.scalar.activation(out=mt[:, k_, :], in_=pmt,
                                     func=mybir.ActivationFunctionType.Relu)

            # attended[b, au] = sum_j mt[(b j), au] via 0/1 selector matmul
            patt = ps1.tile([BQ, AU], F32, tag="patt")
            for k_ in range(NKC):
                nc.tensor.matmul(patt, sel[:, k_, :], mt[:, k_, :],
                                 start=(k_ == 0), stop=(k_ == NKC - 1))

            # qt_pre[b, au] = q_aug^T @ Wq_aug  (bias folded in)
            pqt = ps1.tile([BQ, AU], F32, tag="pqt")
            for c in range(DC5):
                nc.tensor.matmul(pqt, qr[:, c, :], wq[:, c, :],
                                 start=(c == 0), stop=(c == DC5 - 1))
            qt = pool.tile([BQ, AU], F32)
            nc.scalar.activation(out=qt, in_=pqt,
                                 func=mybir.ActivationFunctionType.Relu)
            x = pool.tile([BQ, AU], F32)
            nc.vector.tensor_add(out=x, in0=patt, in1=qt)

            # layernorm over AU
            stats = pool.tile([BQ, 4], F32)
            nc.vector.tensor_reduce(out=stats[:, 0:1], in_=x,
                                    axis=mybir.AxisListType.X,
                                    op=mybir.AluOpType.add)
            nc.scalar.mul(out=stats[:, 1:2], in_=stats[:, 0:1], mul=-1.0 / AU)
            xc = pool.tile([BQ, AU], F32)
            nc.vector.tensor_scalar_add(out=xc, in0=x, scalar1=stats[:, 1:2])
            sq = pool.tile([BQ, AU], F32)
            nc.scalar.activation(out=sq, in_=xc,
                                 func=mybir.ActivationFunctionType.Square,
                                 accum_out=stats[:, 2:3])
            eps = pool.tile([BQ, 1], F32)
            nc.vector.memset(eps, EPS_LN)
            nc.scalar.activation(out=stats[:, 3:4], in_=stats[:, 2:3],
                                 func=mybir.ActivationFunctionType.Sqrt,
                                 bias=eps, scale=1.0 / AU)
            rstd = pool.tile([BQ, 1], F32)
            nc.vector.reciprocal(out=rstd, in_=stats[:, 3:4])
            nc.vector.tensor_scalar_mul(out=xc, in0=xc, scalar1=rstd)
            nc.vector.tensor_mul(out=xc, in0=xc, in1=grow)
            nc.vector.tensor_add(out=xc, in0=xc, in1=brow)

            # transpose ma -> maT [au, b] (bf16 for the output matmul)
            maT = pool.tile([128, 2, BQ], BF16)
            for a in range(2):
                pst = ps1.tile([128, BQ], F32, tag="pst")
                nc.tensor.transpose(pst, xc[:, a * 128:(a + 1) * 128],
                                    idt[:BQ, :BQ])
                nc.vector.tensor_copy(out=maT[:, a, :], in_=pst)

            # out = [q, ma] @ Wc + bc (bias-aug chunk 4 pairs with qr chunk 4)
            po = ps1.tile([BQ, AU], F32, tag="po")
            for c in range(DC5):
                nc.tensor.matmul(po, qr[:, c, :], wc[:, c, :],
                                 start=(c == 0), stop=False)
            for a in range(2):
                nc.tensor.matmul(po, maT[:, a, :], wc[:, 5 + a, :],
                                 start=False, stop=(a == 1))
            ot = pool.tile([BQ, C], F32)
            nc.vector.tensor_copy(out=ot, in_=po[:, :C])
            nc.sync.dma_start(out=out[:, :], in_=ot)
    nc.finalize()
    return nc


# ---------------------------------------------------------------------------
# SPMD runner with a persistent jitted executable
# ---------------------------------------------------------------------------


class _SpmdRunner:
    def __init__(self, nc, n_cores=NCORES):
        import jax
        from jax.sharding import Mesh, PartitionSpec
        from concourse.bass2jax import (
            _bass_exec_p,
            install_neuronx_cc_hook,
            partition_id_tensor,
        )

        try:
            from jax.experimental.shard_map import shard_map
        except ImportError:
            from jax.shard_map import shard_map

        install_neuronx_cc_hook()
        self.jax = jax
        partition_name = (
            nc.partition_id_tensor.name if nc.partition_id_tensor else None
        )
        in_names, out_names, out_avals, zero_outs = [], [], [], []
        for alloc in nc.m.functions[0].allocations:
            if not isinstance(alloc, mybir.MemoryLocationSet):
                continue
            name = alloc.memorylocations[0].name
            if alloc.kind == "ExternalInput":
                if name != partition_name:
                    in_names.append(name)
            elif alloc.kind == "ExternalOutput":
                shape = tuple(alloc.tensor_shape)
                dtype = mybir.dt.np(alloc.dtype)
                out_names.append(name)
                out_avals.append(jax.core.ShapedArray(shape, dtype))
                zero_outs.append(np.zeros((n_cores * shape[0], *shape[1:]), dtype))
        self.in_names = list(in_names)
        self.out_names = out_names
        self.out_avals = out_avals
        self.zero_outs = zero_outs
        self.n_cores = n_cores
        n_params = len(in_names)
        n_outs = len(out_names)
        all_in = in_names + out_names + ([partition_name] if partition_name else [])

        def _body(*args):
            operands = list(args)
            if partition_name is not None:
                operands.append(partition_id_tensor())
            return tuple(
                _bass_exec_p.bind(
                    *operands,
                    out_avals=tuple(out_avals),
                    in_names=tuple(all_in),
                    out_names=tuple(out_names),
                    lowering_input_output_aliases=(),
                    sim_require_finite=True,
                    sim_require_nnan=True,
                    nc=nc,
                )
            )

        devices = jax.devices()[:n_cores]
        mesh = Mesh(np.asarray(devices), ("core",))
        in_specs = (PartitionSpec("core"),) * (n_params + n_outs)
        out_specs = (PartitionSpec("core"),) * n_outs
        self.sharded = jax.jit(
            shard_map(
                _body, mesh=mesh, in_specs=in_specs, out_specs=out_specs,
                check_rep=False,
            ),
            donate_argnums=tuple(range(n_params, n_params + n_outs)),
            keep_unused=True,
        )

    def __call__(self, concat_in):
        args = [concat_in[n] for n in self.in_names]
        zeros = [np.zeros_like(z) for z in self.zero_outs]
        out_arrs = self.sharded(*args, *zeros)
        res = []
        for c in range(self.n_cores):
            res.append({
                name: np.asarray(out_arrs[i]).reshape(
                    self.n_cores, *self.out_avals[i].shape
                )[c]
                for i, name in enumerate(self.out_names)
            })
        return res


def _rep(a):
    a = np.ascontiguousarray(a)
    return np.broadcast_to(a, (NCORES,) + a.shape).reshape(
        NCORES * a.shape[0], *a.shape[1:]
    )


# ---------------------------------------------------------------------------
# Host orchestration
# ---------------------------------------------------------------------------


def kernel(**inputs):
    qe = np.asarray(inputs["query_embedding"], dtype=np.float32)
    keys = np.asarray(inputs["memory_keys"], dtype=np.float32)
    Wq = np.asarray(inputs["Wq"], dtype=np.float32)
    bq = np.asarray(inputs["bq"], dtype=np.float32)
    Wm = np.asarray(inputs["Wm"], dtype=np.float32)
    bm = np.asarray(inputs["bm"], dtype=np.float32)
    gam = np.asarray(inputs["ln_gamma"], dtype=np.float32)
    bet = np.asarray(inputs["ln_beta"], dtype=np.float32)
    Wc = np.asarray(inputs["Wc"], dtype=np.float32)
    bc_ = np.asarray(inputs["bc"], dtype=np.float32)
    k = int(inputs["k"])
    assert k == K and qe.shape == (B, D) and keys.shape == (N, D)

    # ---- host prep: normalize keys, fp8 layouts ----
    mn = np.sqrt(np.einsum("nd,nd->n", keys, keys, dtype=np.float64)).astype(np.float32)
    kn = keys * (1.0 / mn)[:, None]                 # [N, D] fp32, for rescoring
    qr_full = np.maximum(qe, 0.0)                   # [B, D] fp32 relu'd queries

    k8 = (kn * KSCALE).astype(F8NP)                 # [N, D] fp8
    q8T = np.ascontiguousarray(qr_full.T).astype(F8NP)   # [D, B] fp8
    q8_dev = q8T.reshape(2, 2, 128, B).transpose(2, 0, 1, 3)  # [128,2,2,B]

    import jax
    from jax.sharding import Mesh, NamedSharding, PartitionSpec
    devices = jax.devices()[:NCORES]
    mesh = Mesh(np.asarray(devices), ("core",))
    csh = NamedSharding(mesh, PartitionSpec("core"))

    parts = []
    for c in range(NCORES):
        kT = np.zeros((D, SHP), F8NP)
        kT[:, :SH] = k8[c * SH:(c + 1) * SH].T
        shard = np.ascontiguousarray(
            kT.reshape(2, 2, 128, SHP).transpose(2, 0, 1, 3))
        parts.append(jax.device_put(shard, devices[c]))
    k8_dev = jax.make_array_from_single_device_arrays(
        (NCORES * 128, 2, 2, SHP), csh, parts)

    if "r1" not in _cache:
        _cache["r1"] = _SpmdRunner(_build_phase1())
    res1 = _cache["r1"]({"q8": _rep(np.ascontiguousarray(q8_dev)), "k8": k8_dev})

    # ---- host merge: decode candidates, exact rescore, top-32 ----
    NG = NU * 8                                     # 56 candidate groups/core
    vals = np.empty((B, NCORES, NG), np.float32)
    gidx = np.empty((B, NCORES, NG), np.int64)      # group code: u*256 + g
    ucode = (np.arange(NU, dtype=np.int64) * 256).repeat(8)[None, :]
    for c in range(NCORES):
        t = res1[c]["t8"].reshape(2 * 128, NG).astype(np.float32)
        i = res1[c]["i8"].reshape(2 * 128, NG).astype(np.int64)
        vals[:, c, :] = t
        gidx[:, c, :] = i + ucode

    fvals = vals.reshape(B, NCORES * NG)
    fcore = np.broadcast_to(np.arange(NCORES)[None, :, None],
                            (B, NCORES, NG)).reshape(B, NCORES * NG)
    fgidx = gidx.reshape(B, NCORES * NG)

    top = np.argpartition(-fvals, TOPG - 1, axis=1)[:, :TOPG]   # [B, TOPG]
    tcore = np.take_along_axis(fcore, top, axis=1)
    tg = np.take_along_axis(fgidx, top, axis=1)
    tu, tgg = tg >> 8, tg & 255
    # member columns within shard: unit base + g + stride*m
    stride = np.where(tu < 6, 256, 128)
    cols = (tu * UNIT)[..., None] + tgg[..., None] + \
        stride[..., None] * np.arange(G)[None, None, :]          # [B, TOPG, G]
    valid = cols < SH
    grow_ = tcore[..., None] * SH + np.where(valid, cols, 0)     # [B, TOPG, G]

    cand_rows = grow_.reshape(B, TOPG * G)
    cand_valid = valid.reshape(B, TOPG * G)

    # exact rescore (fp32): sims = kn[rows] . qr  (chunked over queries)
    sims = np.full((B, TOPG * G), -np.inf, np.float32)
    CH = 64
    for lo in range(0, B, CH):
        hi = lo + CH
        kr = kn[cand_rows[lo:hi]]                                # [CH, T*G, D]
        sims[lo:hi] = np.einsum("qkd,qd->qk", kr, qr_full[lo:hi],
                                optimize=True)
    sims[~cand_valid] = -np.inf

    # tie rescue: a duplicated index among a unit's 8 slots near the cut
    # means fp16 value ties may have hidden a distinct group -> rescore unit.
    i8all = gidx                                    # [B, NCORES, NU*8]
    rescued = {}
    v48 = -np.sort(-fvals, axis=1)[:, TOPG - 1]
    for qi, ci, ui in zip(*_find_dup_units(i8all, vals, v48)):
        rows_lo = ci * SH + ui * UNIT
        rows_hi = min(rows_lo + UNIT, ci * SH + SH)
        if rows_hi <= rows_lo:
            continue
        rws = np.arange(rows_lo, rows_hi)
        s = kn[rws] @ qr_full[qi]
        rescued.setdefault(qi, []).append((rws, s))

    top_idx = np.empty((B, K), np.int64)
    order = np.argpartition(-sims, K - 1, axis=1)[:, :K]
    for qi in range(B):
        if qi in rescued:
            rws = np.concatenate([cand_rows[qi]] + [r for r, _ in rescued[qi]])
            svs = np.concatenate([sims[qi]] + [s for _, s in rescued[qi]])
            uniq, first = np.unique(rws, return_index=True)
            svals = np.full(uniq.shape, -np.inf, np.float32)
            np.maximum.at(svals, np.searchsorted(uniq, rws), svs)
            sel = np.argpartition(-svals, K - 1)[:K]
            top_idx[qi] = uniq[sel]
        else:
            top_idx[qi] = np.take_along_axis(cand_rows[qi], order[qi], 0)

    # ---- phase 2 ----
    if "r2" not in _cache:
        _cache["r2"] = _SpmdRunner(_build_phase2())
    r2 = _cache["r2"]

    def _aug_weights(Wmat, bvec):
        a = np.zeros((DC5 * 128, AU), np.float32)
        a[:D] = Wmat
        a[D] = bvec
        return a.reshape(DC5, 128, AU).transpose(1, 0, 2).astype(BF16NP)

    wq_a = _aug_weights(Wq, bq)
    wm_a = _aug_weights(Wm, bm)

    wc_a = np.zeros((7 * 128, AU), np.float32)
    wc_a[:512, :C] = Wc[:512]
    wc_a[512, :C] = bc_
    wc_a[640:896, :C] = Wc[512:768]
    wc_a = wc_a.reshape(7, 128, AU).transpose(1, 0, 2).astype(BF16NP)

    sel = np.zeros((128, NKC, BQ), np.float32)
    p_ = np.arange(128)
    for c in range(NKC):
        sel[p_, c, 4 * c + p_ // 32] = 1.0
    sel = sel.astype(BF16NP)

    mk_cc = np.empty((NCORES, 128, DC5, NK), BF16NP)
    qr_cc = np.empty((NCORES, 128, DC5, BQ), BF16NP)
    for c in range(NCORES):
        flat = top_idx[c * BQ:(c + 1) * BQ].reshape(NK)
        mk_aug = np.zeros((DC5 * 128, NK), np.float32)
        mk_aug[:D] = keys[flat].T
        mk_aug[D] = 1.0
        mk_cc[c] = mk_aug.reshape(DC5, 128, NK).transpose(1, 0, 2).astype(BF16NP)
        q_aug = np.zeros((DC5 * 128, BQ), np.float32)
        q_aug[:D] = qr_full[c * BQ:(c + 1) * BQ].T
        q_aug[D] = 1.0
        qr_cc[c] = q_aug.reshape(DC5, 128, BQ).transpose(1, 0, 2).astype(BF16NP)

    res2 = r2({
        "mk": mk_cc.reshape(NCORES * 128, DC5, NK),
        "qr": qr_cc.reshape(NCORES * 128, DC5, BQ),
        "wq": _rep(wq_a), "wm": _rep(wm_a), "wc": _rep(wc_a),
        "sel": _rep(sel), "gam": _rep(gam), "bet": _rep(bet),
        "ident": _rep(np.eye(128, dtype=np.float32)),
    })

    out = np.concatenate([res2[c]["out"] for c in range(NCORES)], axis=0)
    return out.astype(np.float32)


def _find_dup_units(gidx, vals, v48):
    """(q, core, unit) triples whose 8 slots contain a duplicated index with
    value above the rescore cut (v48 - margin)."""
    Bq, NC, _ = gidx.shape
    g = gidx.reshape(Bq, NC, NU, 8)
    v = vals.reshape(Bq, NC, NU, 8)
    gs = np.sort(g, axis=3)
    dup = (np.diff(gs, axis=3) == 0).any(axis=3)          # [B, NC, NU]
    vmax = v.max(axis=3)
    margin = 8.0
    hit = dup & (vmax >= (v48[:, None, None] - margin))
    return np.nonzero(hit)
